# revision 1
# baseline (speedup 1.0000x reference)
"""ConsensusAttention Trainium2 kernel.

Full-input contract: kernel(levels, non_local_mask) -> out, shapes
  levels:         (8, 1024, 6, 512) float32
  non_local_mask: (1024, 1024) bool   (True = masked out)
  out:            (8, 1024, 6, 512) float32

Sharding: data-parallel over batch (8 cores, one batch element each).

Math per batch element, per level l:
  X = levels[:, l, :]                        (n=1024, d=512)
  r[j] = 1 / (sqrt(d) * ||X_j||)
  S[i, j] = <X_i, X_j> * r[j]
  A = softmax_j(S masked)                    (mask is a local-window mask)
  out[:, l, :] = A @ X

The mask only admits keys with |j - i| <= 96, so for each 256-query
superblock q only key-tiles 2q-1..2q+2 (128 wide, clamped to [0,7])
participate. Scores are computed transposed (S^T[j, i]) so the per-key
scale r[j] rides the ACT exp as a per-partition scale and the exp tiles
feed the output matmul directly as stationary operands (no attention
transposes). Scores are O(1) (|S| <= ||X_i||/sqrt(d) ~ 1.1) so softmax
needs no max-shift; masking is an exact multiply by a 0/1 mask after
exp. Row sums ride N=2 ones-matmuls into PSUM.

Matmuls run in float32r (~tf32, full PE rate at moving dim >= 256; HW
requires producers to write f32r-typed outputs — the DRAM input is
declared f32r (bit-identical) and the transpose/mask copies round).
"""

import sys

sys.path.insert(0, "/opt/trn_rl_repo")

import numpy as np

import concourse.bacc as bacc
import concourse.tile as tile
from concourse import mybir
from concourse.masks import make_identity
from concourse.bass_utils import run_bass_kernel_spmd

B, N, L, D = 8, 1024, 6, 512
NT = N // 128   # 8 key tiles
DC = D // 128   # 4 contraction chunks
NQ = 4          # 256-query superblocks
F32 = mybir.dt.float32
F32R = mybir.dt.float32r


def _tiles(q):
    # key tiles with any unmasked entry for query superblock q
    return list(range(max(2 * q - 1, 0), min(2 * q + 2, NT - 1) + 1))


def _jlo(q):
    # start tile of the (up to 512-wide) mask window staged for q
    return min(max(2 * q - 1, 0), NT - 4)





def _build_nc():
    nc = bacc.Bacc(
        "TRN2",
        target_bir_lowering=False,
        debug=False,
        enable_asserts=True,
        num_devices=8,
    )
    # lv is declared f32r: bit-identical to the f32 numpy input, and lets the
    # DMA land X directly in matmul-legal tiles (PE rounds on read).
    lv = nc.dram_tensor("lv", [N, L, D], F32R, kind="ExternalInput").ap()
    m01 = nc.dram_tensor(
        "m01", [NQ, 512, 256], mybir.dt.bfloat16, kind="ExternalInput"
    ).ap()
    out = nc.dram_tensor("out", [N, L, D], F32, kind="ExternalOutput").ap()

    with tile.TileContext(nc) as tc:
        with (
            tc.tile_pool(name="singles", bufs=1) as singles,
            tc.tile_pool(name="xn_p", bufs=3) as xn_p,
            tc.tile_pool(name="xt_p", bufs=2) as xt_p,
            tc.tile_pool(name="sq_p", bufs=4) as sq_p,
            tc.tile_pool(name="r_p", bufs=2) as r_p,
            tc.tile_pool(name="small_p", bufs=8) as small_p,
            tc.tile_pool(name="e0_p", bufs=6) as e0_p,
            tc.tile_pool(name="et_p", bufs=14) as et_p,
            tc.tile_pool(name="ob_p", bufs=4) as ob_p,
            tc.tile_pool(name="pt_p", bufs=3, space="PSUM") as pt_p,
            tc.tile_pool(name="ps_p", bufs=3, space="PSUM") as ps_p,
            tc.tile_pool(name="po_p", bufs=1, space="PSUM") as po_p,
            tc.tile_pool(name="ss_p", bufs=1, space="PSUM") as ss_p,
        ):
            ident = singles.tile([128, 128], F32)
            make_identity(nc, ident)
            ones_f32 = singles.tile([128, 2], F32)
            nc.vector.memset(ones_f32, 1.0)
            ones2 = singles.tile([128, 2], F32R)
            nc.scalar.copy(out=ones2, in_=ones_f32)
            m01_sb = singles.tile([128, NQ, 4, 256], mybir.dt.bfloat16)

            for l in range(L):
                xn = xn_p.tile([128, NT, D], F32R)
                for c in range(NT):
                    nc.sync.dma_start(
                        out=xn[:, c, :],
                        in_=lv[c * 128 : (c + 1) * 128, l, :],
                    )

                # r[j] = 1/sqrt(D * sum(X_j^2)), one column per key tile
                # (square on the otherwise-idle GPSIMD, reduce on DVE)
                rt = r_p.tile([128, NT], F32)
                r_all = r_p.tile([128, NT], F32)
                if l == 0:
                    # after the level-0 X loads so they win the DMA engines
                    nc.sync.dma_start(
                        out=m01_sb, in_=m01.rearrange("q (t p) i -> p q t i", p=128)
                    )
                nrm = r_p.tile([128, NT], F32)
                for jt in range(NT):
                    sq = sq_p.tile([128, D], F32)
                    nc.gpsimd.tensor_mul(out=sq, in0=xn[:, jt, :], in1=xn[:, jt, :])
                    nc.vector.reduce_sum(
                        out=rt[:, jt : jt + 1], in_=sq, axis=mybir.AxisListType.X
                    )
                nc.scalar.activation(
                    out=nrm, in_=rt, func=mybir.ActivationFunctionType.Sqrt,
                    scale=float(D),
                )
                nc.vector.reciprocal(out=r_all, in_=nrm)

                # X^T via PE transposes: xt[pd, dc, j] = X[j, dc*128+pd].
                # 4 dc-chunks share one PSUM bank; one batched copy per tile.
                xt = xt_p.tile([128, DC, N], F32R)
                for jt in range(NT):
                    pt = pt_p.tile([128, DC, 128], F32)
                    for dc in range(DC):
                        nc.tensor.transpose(
                            out=pt[:, dc, :],
                            in_=xn[:, jt, dc * 128 : (dc + 1) * 128].bitcast(F32),
                            identity=ident,
                        )
                    dst = xt[:, :, jt * 128 : (jt + 1) * 128]
                    if jt % 4 == 0:
                        nc.scalar.copy(out=dst, in_=pt)
                    else:
                        nc.vector.tensor_copy(out=dst, in_=pt)

                for q in range(NQ):
                    jlo = _jlo(q)
                    tl = _tiles(q)
                    qs = slice(q * 256, (q + 1) * 256)
                    ets = {}
                    for jt in tl:
                        ps = ps_p.tile([128, 256], F32)
                        for dc in range(DC):
                            nc.tensor.matmul(
                                ps,
                                lhsT=xt[:, dc, jt * 128 : (jt + 1) * 128],
                                rhs=xt[:, dc, qs],
                                start=(dc == 0),
                                stop=(dc == DC - 1),
                            )
                        e0 = e0_p.tile([128, 256], F32)
                        nc.scalar.activation(
                            out=e0,
                            in_=ps,
                            func=mybir.ActivationFunctionType.Exp,
                            scale=r_all[:, jt : jt + 1],
                        )
                        et = et_p.tile([128, 256], F32R)
                        nc.vector.tensor_mul(
                            out=et, in0=e0, in1=m01_sb[:, q, jt - jlo, :]
                        )
                        ets[jt] = et

                    ss = ss_p.tile([128, 4], F32)
                    ob = ob_p.tile([128, 2, D], F32)
                    for h in range(2):
                        po = po_p.tile([128, D], F32)
                        for i, jt in enumerate(tl):
                            eh = ets[jt][:, h * 128 : (h + 1) * 128]
                            nc.tensor.matmul(
                                po,
                                lhsT=eh,
                                rhs=xn[:, jt, :],
                                start=(i == 0),
                                stop=(i == len(tl) - 1),
                            )
                            nc.tensor.matmul(
                                ss[:, 2 * h : 2 * h + 2],
                                lhsT=eh,
                                rhs=ones2,
                                start=(i == 0),
                                stop=(i == len(tl) - 1),
                            )
                        rec = small_p.tile([128, 1], F32)
                        nc.vector.reciprocal(out=rec, in_=ss[:, 2 * h : 2 * h + 1])
                        if h == 0:
                            nc.scalar.activation(
                                out=ob[:, 0, :],
                                in_=po,
                                func=mybir.ActivationFunctionType.Copy,
                                scale=rec,
                            )
                        else:
                            nc.vector.tensor_scalar_mul(
                                out=ob[:, 1, :], in0=po, scalar1=rec
                            )
                    for h2 in range(2):
                        nc.sync.dma_start(
                            out=out[q * 256 + h2 * 128 : q * 256 + (h2 + 1) * 128, l, :],
                            in_=ob[:, h2, :],
                        )

    nc.compile()
    return nc


_NC = None


def get_nc():
    global _NC
    if _NC is None:
        _NC = _build_nc()
    return _NC


def _band_ok(mask):
    # every unmasked (i, j) must fall inside q's staged key tiles
    for q in range(NQ):
        rows = ~mask[q * 256 : (q + 1) * 256, :]
        outside = np.ones(N, dtype=bool)
        for jt in _tiles(q):
            outside[jt * 128 : (jt + 1) * 128] = False
        if rows[:, outside].any():
            return False
    # no all-masked row (softmax denominator would be 0)
    if (~mask).sum(axis=1).min() == 0:
        return False
    return True


def _numpy_ref(levels, mask):
    levels = levels.astype(np.float32)
    nrm = np.linalg.norm(levels, axis=-1, keepdims=True)
    k = levels / np.maximum(nrm, 1e-12)
    sim = np.einsum("bild,bjld->blij", levels, k) * (levels.shape[-1] ** -0.5)
    sim = np.where(mask[None, None, :, :], -np.finfo(np.float32).max, sim)
    sim = sim - sim.max(axis=-1, keepdims=True)
    e = np.exp(sim)
    attn = e / e.sum(axis=-1, keepdims=True)
    return np.einsum("blij,bjld->bild", attn, levels).astype(np.float32)


def kernel(levels, non_local_mask):
    levels = np.ascontiguousarray(levels, dtype=np.float32)
    mask = np.asarray(non_local_mask).astype(bool)
    if levels.shape != (B, N, L, D) or mask.shape != (N, N) or not _band_ok(mask):
        return _numpy_ref(levels, mask)

    m01 = np.zeros((NQ, 512, 256), dtype=np.float32)
    for q in range(NQ):
        jlo = _jlo(q)
        w = (~mask[q * 256 : (q + 1) * 256, jlo * 128 : jlo * 128 + 512]).T
        m01[q] = w.astype(np.float32)

    import ml_dtypes

    m01 = m01.astype(ml_dtypes.bfloat16)
    nc = get_nc()
    in_maps = [{"lv": levels[b], "m01": m01} for b in range(B)]
    res = run_bass_kernel_spmd(nc, in_maps, core_ids=list(range(B)))
    return np.stack([res.results[b]["out"] for b in range(B)])



# revision 16
# speedup vs baseline: 1.1389x; 1.1389x over previous
"""ConsensusAttention Trainium2 kernel (v2).

Full-input contract: kernel(levels, non_local_mask) -> out, shapes
  levels:         (8, 1024, 6, 512) float32
  non_local_mask: (1024, 1024) bool   (True = masked out)
  out:            (8, 1024, 6, 512) float32

Sharding: data-parallel over batch (8 cores, one batch element each).

Math per batch element, per level l:
  X = levels[:, l, :]                        (n=1024, d=512)
  r[j] = 1 / (sqrt(d) * ||X_j||)
  S[i, j] = <X_i, X_j> * r[j]
  A = softmax_j(S masked)                    (radius-3 disc on a 32x32 grid)
  out[:, l, :] = A @ X

Scores are O(1) so softmax needs no max-shift. Scores are computed
transposed (S^T[j, i], keys on partitions) so r[j] rides the ACT exp as
a per-partition scale and the exp tiles feed the output matmul directly
as stationary operands.

Per-level device pipeline (one NeuronCore does its batch's 6 levels):
  - two batched DMAs load X as bf16 (matmul values) and fp8e4 (score
    operands), [128, 8, 512] each
  - PE transposes the fp8 X (fp8 identity -> 1 cycle/row) into fp8 PSUM;
    the PSUM->SBUF copyback is dtype-preserving so it rides the DMA
    engine (4 batched pair-DMAs) instead of ACT/DVE
  - ||X_j||^2 via DVE scalar_tensor_tensor square + row-accumulate
  - S^T per key tile over a 256/512-query window: fp8e4 DoubleRow
    matmuls (2 passes of 256-contraction, 0.5 cycles/row), then the
    boolean mask is ADDED in-PSUM as a -57344 bias via one fp8e5
    DoubleRow matmul with an [I;0]/[0;I] stationary (exp of a masked
    entry is exp(~-100) = 0, so no separate mask multiply)
  - ACT exp (scale = r[j]) emits masked exp tiles directly in bf16
  - output matmul in bf16 (exact values): per 128-query block only the
    3 key tiles the disc mask can reach; row sums ride tiny
    ones-matmuls into PSUM; ACT/DVE normalize into a bf16 staging tile
  - one batched DMA stores the level's output (bf16, host widens)

Levels are software-pipelined: level l+1's transposes are emitted
before level l's output matmuls so the PE never waits on ACT exp.
"""

import sys

sys.path.insert(0, "/opt/trn_rl_repo")

import numpy as np

import concourse.bacc as bacc
import concourse.tile as tile
from concourse import mybir
from concourse.masks import make_identity
from concourse.bass_utils import run_bass_kernel_spmd

B, N, L, D = 8, 1024, 6, 512
NT = N // 128   # 8 key tiles
DC = D // 128   # 4 contraction chunks
NH = 8          # 128-query half-blocks
F32 = mybir.dt.float32
BF16 = mybir.dt.bfloat16
F8E4 = mybir.dt.float8e4
F8E5 = mybir.dt.float8e5
U16 = mybir.dt.uint16
DR = mybir.MatmulPerfMode.DoubleRow

MASK_BIAS = -57344.0  # exactly representable in fp8e5; r*57344 ~ 100 >> ln-range

# per key tile jt: query window [W0[jt], W0[jt]+WW[jt]) that can reach it
W0 = [0, 0, 0, 256, 256, 512, 512, 768]
WW = [256, 512, 512, 512, 512, 512, 512, 256]


def _htiles(h):
    # key tiles with any unmasked entry for 128-query half-block h
    return list(range(max(h - 1, 0), min(h + 1, NT - 1) + 1))


def _build_nc():
    nc = bacc.Bacc(
        "TRN2",
        target_bir_lowering=False,
        debug=False,
        enable_asserts=True,
        num_devices=8,
    )
    lv = nc.dram_tensor("lv", [N, L, D], BF16, kind="ExternalInput").ap()
    lv8 = nc.dram_tensor("lv8", [N, L, D], F8E4, kind="ExternalInput").ap()
    m01 = nc.dram_tensor("m01", [128, NT, 512], F8E5, kind="ExternalInput").ap()
    out = nc.dram_tensor("out", [N, L, D], BF16, kind="ExternalOutput").ap()

    lv_r = lv.rearrange("(t p) l d -> p t l d", p=128)
    lv8_r = lv8.rearrange("(t p) l d -> p t l d", p=128)
    out_r = out.rearrange("(h p) l d -> p h l d", p=128)

    with tile.TileContext(nc) as tc:
        with (
            tc.tile_pool(name="singles", bufs=1) as singles,
            tc.tile_pool(name="xn_p", bufs=2) as xn_p,
            tc.tile_pool(name="xn8_p", bufs=2) as xn8_p,
            tc.tile_pool(name="xt_p", bufs=2) as xt_p,
            tc.tile_pool(name="sq_p", bufs=2) as sq_p,
            tc.tile_pool(name="r_p", bufs=6) as r_p,
            tc.tile_pool(name="et_p", bufs=2) as et_p,
            tc.tile_pool(name="ob_p", bufs=2) as ob_p,
            tc.tile_pool(name="rec_p", bufs=16) as rec_p,
            tc.tile_pool(name="pt_p", bufs=2, space="PSUM") as pt_p,
            tc.tile_pool(name="s_ps_p", bufs=3, space="PSUM") as s_ps_p,
            tc.tile_pool(name="po_p", bufs=2, space="PSUM") as po_p,
            tc.tile_pool(name="ss_p", bufs=1, space="PSUM") as ss_p,
        ):
            ident = singles.tile([128, 128], F8E4)
            make_identity(nc, ident)
            # [I; 0; I] in fp8e5: slices [0:2] / [1:3] select which half of a
            # DoubleRow rhs pair lands in the PSUM (the other half gets x0)
            id3 = singles.tile([128, 3, 128], F8E5)
            nc.gpsimd.memset(id3, 0.0)
            make_identity(nc, id3[:, 0, :], nomemset=True)
            make_identity(nc, id3[:, 2, :], nomemset=True)
            ones = singles.tile([128, 1], BF16)
            nc.vector.memset(ones, 1.0)
            m01_sb = singles.tile([128, NT, 512], F8E5)

            def load_level(l):
                xn = xn_p.tile([128, NT, D], BF16)
                nc.sync.dma_start(out=xn, in_=lv_r[:, :, l, :])
                xn8 = xn8_p.tile([128, NT, D], F8E4)
                nc.sync.dma_start(out=xn8, in_=lv8_r[:, :, l, :])
                return xn, xn8

            def transpose_norms(l, xn, xn8):
                # X^T in fp8e4. HW fp8 transposes must write with element
                # step 2 from a 4-byte-aligned base, so values live at even
                # bytes of [., 2] pairs (odd bytes are junk). The copyback
                # moves whole uint16 pairs (DVE 2-byte fast path) and the
                # score matmuls read the fp8 values at stride 2.
                xt = xt_p.tile([128, NT, DC, 128, 2], F8E4)
                nrm2 = r_p.tile([128, NT], F32)
                sq = sq_p.tile([128, D], BF16)
                for jt2 in range(NT // 2):
                    pt2 = pt_p.tile([128, 2, DC, 128, 2], F8E4)
                    for k in range(2):
                        jt = 2 * jt2 + k
                        for dc in range(DC):
                            nc.tensor.transpose(
                                out=pt2[:, k, dc, :, 0],
                                in_=xn8[:, jt, dc * 128 : (dc + 1) * 128],
                                identity=ident,
                            )
                        nc.vector.scalar_tensor_tensor(
                            out=sq,
                            in0=xn[:, jt, :],
                            scalar=1.0,
                            in1=xn[:, jt, :],
                            op0=mybir.AluOpType.mult,
                            op1=mybir.AluOpType.mult,
                            accum_out=nrm2[:, jt : jt + 1],
                        )
                    # dtype-preserving copyback; uint16 bitcast enables the
                    # DVE 2-byte fast path (fp8 APs never qualify)
                    nc.vector.tensor_copy(
                        out=xt[:, 2 * jt2 : 2 * jt2 + 2].bitcast(U16),
                        in_=pt2.bitcast(U16),
                    )
                nrm = r_p.tile([128, NT], F32)
                r_all = r_p.tile([128, NT], F32)
                nc.scalar.activation(
                    out=nrm, in_=nrm2, func=mybir.ActivationFunctionType.Sqrt,
                    scale=float(D),
                )
                nc.vector.reciprocal(out=r_all, in_=nrm)
                return xt, r_all

            def scores(l, xt, r_all):
                # masked exp(S^T) per key tile over its query window
                et = et_p.tile([128, NT, 512], BF16)
                for jt in range(NT):
                    w0, ww = W0[jt], WW[jt]
                    lo = jt if jt < NT - 1 else jt - 1
                    sel = 0 if jt < NT - 1 else 1
                    ps = s_ps_p.tile([128, 512], F32)
                    for t in range(ww // 128):
                        qt = w0 // 128 + t
                        reg = ps[:, t * 128 : (t + 1) * 128]
                        for c in range(2):
                            nc.tensor.matmul(
                                reg,
                                lhsT=xt[:, jt, 2 * c : 2 * c + 2, :, 0],
                                rhs=xt[:, qt, 2 * c : 2 * c + 2, :, 0],
                                start=(c == 0),
                                stop=False,
                                perf_mode=DR,
                            )
                        # += mask bias: [I;0] (or [0;I] for the last key
                        # tile) picks slot jt out of the (lo, lo+1) rhs pair
                        nc.tensor.matmul(
                            reg,
                            lhsT=id3[:, sel : sel + 2, :],
                            rhs=m01_sb[:, lo : lo + 2, t * 128 : (t + 1) * 128],
                            start=False,
                            stop=True,
                            perf_mode=DR,
                        )
                    nc.scalar.activation(
                        out=et[:, jt, :ww],
                        in_=ps[:, :ww],
                        func=mybir.ActivationFunctionType.Exp,
                        scale=r_all[:, jt : jt + 1],
                    )
                return et

            def outputs(l, xn, et):
                # out[i,:] = sum_j A^T[j,i] X[j,:] over the 3 reachable tiles
                ob = ob_p.tile([128, NH, D], BF16)
                ss = ss_p.tile([128, NH], F32)
                for h in range(NH):
                    tl = _htiles(h)
                    po = po_p.tile([128, D], F32)
                    for i, jt in enumerate(tl):
                        eh = et[:, jt, 128 * h - W0[jt] : 128 * h - W0[jt] + 128]
                        nc.tensor.matmul(
                            po,
                            lhsT=eh,
                            rhs=xn[:, jt, :],
                            start=(i == 0),
                            stop=(i == len(tl) - 1),
                        )
                        nc.tensor.matmul(
                            ss[:, h : h + 1],
                            lhsT=eh,
                            rhs=ones,
                            start=(i == 0),
                            stop=(i == len(tl) - 1),
                        )
                    rec = rec_p.tile([128, 1], F32)
                    nc.vector.reciprocal(out=rec, in_=ss[:, h : h + 1])
                    if h % 4 != 3:
                        nc.scalar.activation(
                            out=ob[:, h, :],
                            in_=po,
                            func=mybir.ActivationFunctionType.Copy,
                            scale=rec,
                        )
                    else:
                        nc.vector.tensor_scalar_mul(
                            out=ob[:, h, :], in0=po, scalar1=rec
                        )
                nc.sync.dma_start(out=out_r[:, :, l, :], in_=ob)

            # software pipeline: next level's transposes run on PE before the
            # previous level's output matmuls wait on ACT exp tiles
            xn, xn8 = load_level(0)
            xt, r_all = transpose_norms(0, xn, xn8)
            nc.sync.dma_start(out=m01_sb, in_=m01)
            for l in range(L):
                et = scores(l, xt, r_all)
                if l + 1 < L:
                    xn_next, xn8_next = load_level(l + 1)
                    xt_next, r_next = transpose_norms(l + 1, xn_next, xn8_next)
                else:
                    xn_next = xt_next = r_next = None
                outputs(l, xn, et)
                xn, xt, r_all = xn_next, xt_next, r_next

    nc.compile()
    return nc


_NC = None


def get_nc():
    global _NC
    if _NC is None:
        _NC = _build_nc()
    return _NC


def _band_ok(mask):
    # every unmasked (i, j) must fall inside jt's staged query window and
    # inside the 3-tile key window of i's half-block; no all-masked row
    unm = ~mask
    for jt in range(NT):
        cols = unm[:, jt * 128 : (jt + 1) * 128]
        rows = np.zeros(N, dtype=bool)
        rows[W0[jt] : W0[jt] + WW[jt]] = True
        if cols[~rows, :].any():
            return False
    for h in range(NH):
        rows = unm[h * 128 : (h + 1) * 128, :]
        outside = np.ones(N, dtype=bool)
        for jt in _htiles(h):
            outside[jt * 128 : (jt + 1) * 128] = False
        if rows[:, outside].any():
            return False
    if unm.sum(axis=1).min() == 0:
        return False
    return True


def _numpy_ref(levels, mask):
    levels = levels.astype(np.float32)
    nrm = np.linalg.norm(levels, axis=-1, keepdims=True)
    k = levels / np.maximum(nrm, 1e-12)
    sim = np.einsum("bild,bjld->blij", levels, k) * (levels.shape[-1] ** -0.5)
    sim = np.where(mask[None, None, :, :], -np.finfo(np.float32).max, sim)
    sim = sim - sim.max(axis=-1, keepdims=True)
    e = np.exp(sim)
    attn = e / e.sum(axis=-1, keepdims=True)
    return np.einsum("blij,bjld->bild", attn, levels).astype(np.float32)


def _build_m01(mask):
    # additive bias, S^T layout: slot [p, jt, f] covers key j=jt*128+p,
    # query i=W0[jt]+f (slots for 256-wide windows leave [256:512] unused)
    import ml_dtypes

    m01 = np.zeros((128, NT, 512), dtype=np.float32)
    for jt in range(NT):
        w0, ww = W0[jt], WW[jt]
        sub = mask[w0 : w0 + ww, jt * 128 : (jt + 1) * 128]  # [i, j]
        m01[:, jt, :ww] = np.where(sub.T, np.float32(MASK_BIAS), np.float32(0.0))
    return m01.astype(ml_dtypes.float8_e5m2)


def kernel(levels, non_local_mask):
    levels = np.ascontiguousarray(levels, dtype=np.float32)
    mask = np.asarray(non_local_mask).astype(bool)
    if levels.shape != (B, N, L, D) or mask.shape != (N, N) or not _band_ok(mask):
        return _numpy_ref(levels, mask)

    import ml_dtypes

    lv16 = levels.astype(ml_dtypes.bfloat16)
    lv8 = levels.astype(ml_dtypes.float8_e4m3)
    m01 = _build_m01(mask)
    nc = get_nc()
    in_maps = [{"lv": lv16[b], "lv8": lv8[b], "m01": m01} for b in range(B)]
    res = run_bass_kernel_spmd(nc, in_maps, core_ids=list(range(B)))
    return np.stack(
        [res.results[b]["out"].astype(np.float32) for b in range(B)]
    )
